# revision 11
# baseline (speedup 1.0000x reference)
"""Trainium2 Bass kernel for nn_Causal_Kron_Block_MLP.

Reference computation (B=4, L=2048, D=1024, H=16, HD=64):
    y1 = x @ W1a.T                                   # [B,L,D]
    z  = relu(einsum('hlm,bhmd->bhld', tril(mat2a), split_heads(y1)))
    y2 = merge_heads(z) @ W1b.T
    w  = einsum('hlm,bhmd->bhld', tril(mat2b), split_heads(y2))
    out = einsum('bhld,hde->ble', w, w_out)

Sharding: 8 cores, head-parallel — core c owns heads (2c, 2c+1).
Each core computes y1/z for its 2 heads over the full batch; an
AllGather (split in two chunks, overlapped with compute) exchanges z
(the only cross-head mixing point is W1b); each core then computes
the y2 columns for its heads, the tril_b stage, and a partial
head-sum of the output; the host sums the 8 partials.

Layouts (device, per core; r = global row index (b, l), R = 8192):
    y1mT/y2mT: per (h_rel, p, m-block) tiles [128 = m, 128 = (j, d)]
               built by PE-transposes fused with stages 1/3
    z_all0/1:  [512 = (rank, d), R] per h_rel chunk (AllGather out)
    wT_sb:     [128 = (h_rel, d), R]  (stage-4 output, reassembled)
    out_part:  [R, D] f32             (partial head-sum, DRAM)

All matmul operands are float32r (fp32 rounded to 11 mantissa bits;
~236 ns per [128x128]x[128x512] matmul vs ~920 ns for fp32). Inputs
are pre-rounded on the host; on-device intermediates are rounded by
PSUM->SBUF copies writing float32r. Causality: tril blocks entirely
above the diagonal are never loaded nor multiplied. DMAs are batched
via multi-dim access patterns to keep the Sync sequencer off the
critical path.
"""

import numpy as np

import concourse.bass as bass
import concourse.mybir as mybir
import concourse.tile as tile
from concourse import bacc
from concourse.bass_utils import run_bass_kernel_spmd

B, L, D, H, HD = 4, 2048, 1024, 16, 64
NCORES = 8
R = B * L               # 8192 global rows
NB = 512                # moving free-dim per matmul
N_RB = R // NB          # 16 row-blocks of 512
N_KB_D = D // 128       # 8 k-blocks over model dim
N_MB = L // 128         # 16 m-blocks over seq per batch
N_LB = L // NB          # 4 l-blocks of 512 per batch
MB_G = 4                # tril m-blocks fetched per DMA
F32 = mybir.dt.float32
F32R = mybir.dt.float32r

_NC_CACHE = {}


def round_fp32r(a: np.ndarray) -> np.ndarray:
    """Round-to-nearest-even fp32 -> fp32r (11 mantissa bits kept)."""
    u = np.ascontiguousarray(a, dtype=np.float32).view(np.uint32)
    lsb = (u >> 12) & 1
    r = (u + np.uint32(0x7FF) + lsb) & np.uint32(0xFFFFF000)
    return r.view(np.float32)


def build_nc():
    """Build the single-NEFF SPMD kernel (same program on all 8 cores)."""
    nc = bacc.Bacc(None, target_bir_lowering=False)

    xT = nc.dram_tensor("xT", [D, R], F32R, kind="ExternalInput")
    w1aT = nc.dram_tensor("w1aT", [D, 128], F32R, kind="ExternalInput")
    # w1bT rows are host-permuted to the chunked-AllGather k order:
    # chunk h_rel, then (rank, d).
    w1bT = nc.dram_tensor("w1bT", [D, 128], F32R, kind="ExternalInput")
    trilAT = nc.dram_tensor("trilAT", [2, L, L], F32R, kind="ExternalInput")
    trilBT = nc.dram_tensor("trilBT", [2, L, L], F32R, kind="ExternalInput")
    wout = nc.dram_tensor("wout", [128, D], F32R, kind="ExternalInput")
    ident_in = nc.dram_tensor("ident", [128, 128], F32R, kind="ExternalInput")
    out_part = nc.dram_tensor("out_part", [R, D], F32, kind="ExternalOutput")

    with tile.TileContext(nc) as tc:
        with (
            tc.tile_pool(name="persist", bufs=1) as persist,
            tc.tile_pool(name="psmm", bufs=4, space="PSUM") as psmm,
            tc.tile_pool(name="pstr", bufs=3, space="PSUM") as pstr,
            tc.tile_pool(name="dram", bufs=1, space="DRAM") as dram,
        ):
            ident = persist.tile([128, 128], F32R, tag="ident")
            nc.sync.dma_start(out=ident[:], in_=ident_in[:])

            w1aT_sb = persist.tile([128, D], F32R, tag="w1aT")
            w1bT_sb = persist.tile([128, D], F32R, tag="w1bT")
            wout_sb = persist.tile([128, D], F32R, tag="wout")
            nc.sync.dma_start(
                out=w1aT_sb[:].rearrange("p (g n) -> p g n", g=N_KB_D),
                in_=w1aT[:].rearrange("(g p) n -> p g n", p=128),
            )
            nc.sync.dma_start(
                out=w1bT_sb[:].rearrange("p (g n) -> p g n", g=N_KB_D),
                in_=w1bT[:].rearrange("(g p) n -> p g n", p=128),
            )
            nc.sync.dma_start(out=wout_sb[:], in_=wout[:])

            # AllGather chunks: z_in[h_rel] [HD, R] -> z_all[h_rel] [8*HD, R]
            z_in = [
                dram.tile([HD, R], F32R, tag=f"z_in{h}", name=f"z_in{h}")
                for h in range(2)
            ]
            z_all = [
                dram.tile(
                    [NCORES * HD, R], F32R, tag=f"z_all{h}",
                    name=f"z_all{h}", addr_space="Shared",
                )
                for h in range(2)
            ]

            # -------- stage 1/3 + fused transpose glue ------------------
            def linear_stage(src_load, wT_sb_, dstmT, scope):
                """dstmT tiles [128=m, 128=(j,d)] per (h_rel, p, mb) from
                out[128=(h_rel,d), r] = wT_sb_.T @ src, PE-transposed."""
                with nc.named_scope(scope):
                    for rb in range(N_RB):
                        b, lc = rb // N_LB, rb % N_LB
                        p, j = b // 2, b % 2
                        ps = psmm.tile([128, NB], F32, tag="ps_mm")
                        src = src_load(rb)
                        for kb in range(N_KB_D):
                            nc.tensor.matmul(
                                ps[:],
                                wT_sb_[:, kb * 128 : (kb + 1) * 128],
                                src[:, kb * NB : (kb + 1) * NB],
                                start=(kb == 0),
                                stop=(kb == N_KB_D - 1),
                            )
                        yt = persist.tile(
                            [128, NB], F32R, tag="yt", bufs=4, name=f"yt_{scope}_{rb}"
                        )
                        nc.scalar.activation(
                            yt[:], ps[:], mybir.ActivationFunctionType.Copy
                        )
                        for h_rel in range(2):
                            for ml in range(NB // 128):
                                mb = lc * (NB // 128) + ml
                                pst = pstr.tile([128, HD], F32R, tag="ps_tr")
                                nc.tensor.transpose(
                                    pst[:],
                                    yt[
                                        h_rel * HD : (h_rel + 1) * HD,
                                        ml * 128 : (ml + 1) * 128,
                                    ],
                                    ident[
                                        h_rel * HD : (h_rel + 1) * HD,
                                        h_rel * HD : (h_rel + 1) * HD,
                                    ],
                                )
                                off = ((h_rel * 2 + p) * N_MB + mb) * 128
                                nc.vector.tensor_copy(
                                    dstmT[:, off + j * HD : off + (j + 1) * HD],
                                    pst[:],
                                )

            # -------- stages 2/4: out = y.T @ trilT (causal) ------------
            def tril_stage(trilT, srcmT, out_cb, tpool, scope, h_rel):
                # Full 512-wide m-block groups strictly below the diagonal,
                # then 4 diagonal m-blocks loaded without their zero prefix.
                with nc.named_scope(f"{scope}h{h_rel}"):
                    for lb in range(N_LB):
                        n_mb = (lb + 1) * (NB // 128)
                        pss = [
                            psmm.tile(
                                [128, NB], F32, tag="ps_mm",
                                name=f"ps_{scope}_{h_rel}_{lb}_{pi}",
                            )
                            for pi in range(2)
                        ]
                        for mg in range(0, lb * MB_G, MB_G):
                            tblk = tpool.tile(
                                [128, MB_G * NB], F32R, tag="tril_blk",
                                name=f"tb_{scope}_{h_rel}_{lb}_{mg}",
                            )
                            nc.sync.dma_start(
                                out=tblk[:].rearrange(
                                    "p (g n) -> p g n", g=MB_G
                                ),
                                in_=trilT[
                                    h_rel,
                                    mg * 128 : (mg + MB_G) * 128,
                                    lb * NB : (lb + 1) * NB,
                                ].rearrange("(g p) n -> p g n", p=128),
                            )
                            for mi in range(MB_G):
                                mb = mg + mi
                                for p in range(2):
                                    off = ((h_rel * 2 + p) * N_MB + mb) * 128
                                    nc.tensor.matmul(
                                        pss[p][:],
                                        srcmT[:, off : off + 128],
                                        tblk[:, mi * NB : (mi + 1) * NB],
                                        start=(mb == 0),
                                        stop=False,
                                    )
                        # diagonal group: m-block lb*4+i has i*128 leading zeros
                        for i in range(MB_G):
                            mb = lb * MB_G + i
                            w = NB - i * 128
                            dblk = tpool.tile(
                                [128, NB], F32R, tag="diag_blk",
                                name=f"db_{scope}_{h_rel}_{lb}_{i}",
                            )
                            nc.sync.dma_start(
                                out=dblk[:, :w],
                                in_=trilT[
                                    h_rel,
                                    mb * 128 : (mb + 1) * 128,
                                    lb * NB + i * 128 : (lb + 1) * NB,
                                ],
                            )
                            for p in range(2):
                                off = ((h_rel * 2 + p) * N_MB + mb) * 128
                                nc.tensor.matmul(
                                    pss[p][:, i * 128 : NB],
                                    srcmT[:, off : off + 128],
                                    dblk[:, :w],
                                    start=(mb == 0),
                                    stop=(i == MB_G - 1),
                                )
                        for p in range(2):
                            out_cb(h_rel, p, lb, pss[p])

            # ================= phase A ==================================
            with (
                tc.tile_pool(name="xin", bufs=2) as xin,
                tc.tile_pool(name="trilA_p", bufs=3) as trilA_p,
                tc.tile_pool(name="mtA", bufs=1) as mtA,
            ):
                y1mT = mtA.tile([128, 2 * R], F32R, tag="y1mT")
                z_sb = mtA.tile([128, R], F32R, tag="z_sb")

                def x_load(rb):
                    xt = xin.tile([128, D // 128 * NB], F32R, tag="x_blk",
                                  name=f"x_{rb}")
                    nc.sync.dma_start(
                        out=xt[:].rearrange("p (g n) -> p g n", g=N_KB_D),
                        in_=xT[:, rb * NB : (rb + 1) * NB].rearrange(
                            "(g p) n -> p g n", p=128
                        ),
                    )
                    return xt

                linear_stage(x_load, w1aT_sb, y1mT, "s1")

                def z_out(h_rel, p, lb, ps):
                    base = (h_rel * 2 + p) * L
                    nc.scalar.activation(
                        z_sb[:, base + lb * NB : base + (lb + 1) * NB],
                        ps[:],
                        mybir.ActivationFunctionType.Relu,
                    )

                # per-h_rel: stage 2 chunk -> z_in DMAs -> AllGather, so the
                # first gather overlaps the second chunk's compute
                for h_rel in range(2):
                    tril_stage(trilAT, y1mT, z_out, trilA_p, "s2", h_rel)
                    with nc.named_scope(f"ag_in{h_rel}"):
                        for p in range(2):
                            for j in range(2):
                                b = 2 * p + j
                                nc.sync.dma_start(
                                    out=z_in[h_rel][:, b * L : (b + 1) * L],
                                    in_=z_sb[
                                        j * HD : (j + 1) * HD,
                                        (h_rel * 2 + p) * L
                                        : (h_rel * 2 + p + 1) * L,
                                    ],
                                )
                    nc.gpsimd.collective_compute(
                        "AllGather",
                        mybir.AluOpType.bypass,
                        replica_groups=[list(range(NCORES))],
                        ins=[z_in[h_rel].opt()],
                        outs=[z_all[h_rel].opt()],
                    )

            # ================= phase B ==================================
            with (
                tc.tile_pool(name="zin_p", bufs=2) as zin_p,
                tc.tile_pool(name="trilB_p", bufs=2) as trilB_p,
                tc.tile_pool(name="mtB", bufs=1) as mtB,
                tc.tile_pool(name="stg", bufs=3) as stg,
            ):
                y2mT = mtB.tile([128, 2 * R], F32R, tag="y2mT")
                wT_sb = mtB.tile([128, R], F32R, tag="wT_sb")

                def z_load(rb):
                    # 8 k-blocks: chunk h_rel = kb//4, rows (rank,d)
                    zt = zin_p.tile([128, D // 128 * NB], F32R, tag="z_blk",
                                    name=f"z_{rb}")
                    for h_rel in range(2):
                        nc.sync.dma_start(
                            out=zt[
                                :, h_rel * 4 * NB : (h_rel + 1) * 4 * NB
                            ].rearrange("p (g n) -> p g n", g=4),
                            in_=z_all[h_rel][
                                :, rb * NB : (rb + 1) * NB
                            ].rearrange("(g p) n -> p g n", p=128),
                        )
                    return zt

                linear_stage(z_load, w1bT_sb, y2mT, "s3")

                def w_cb(h_rel, p, lb, ps):
                    st = stg.tile([128, NB], F32R, tag="w_stage",
                                  name=f"wst_{h_rel}_{p}_{lb}")
                    nc.scalar.activation(
                        st[:], ps[:], mybir.ActivationFunctionType.Copy
                    )
                    for j in range(2):
                        b = 2 * p + j
                        nc.sync.dma_start(
                            out=wT_sb[
                                h_rel * HD : (h_rel + 1) * HD,
                                b * L + lb * NB : b * L + (lb + 1) * NB,
                            ],
                            in_=st[j * HD : (j + 1) * HD, :],
                        )

                for h_rel in range(2):
                    tril_stage(trilBT, y2mT, w_cb, trilB_p, "s4", h_rel)

                # stage 5: out_part rows = wT.T @ wout
                with nc.named_scope("s5"):
                    for rb in range(R // 128):
                        ost = stg.tile([128, D], F32, tag="out_stage",
                                       bufs=2, name=f"ost_{rb}")
                        for eh in range(2):
                            ps = psmm.tile([128, NB], F32, tag="ps_mm",
                                           name=f"ps5_{rb}_{eh}")
                            nc.tensor.matmul(
                                ps[:],
                                wT_sb[:, rb * 128 : (rb + 1) * 128],
                                wout_sb[:, eh * NB : (eh + 1) * NB],
                                start=True,
                                stop=True,
                            )
                            nc.scalar.activation(
                                ost[:, eh * NB : (eh + 1) * NB],
                                ps[:],
                                mybir.ActivationFunctionType.Copy,
                            )
                        nc.sync.dma_start(
                            out=out_part[rb * 128 : (rb + 1) * 128, :],
                            in_=ost[:],
                        )

    nc.finalize()
    return nc


def prep_in_maps(x, W1a, W1b, mat2a, mat2b, w_out):
    xT = round_fp32r(np.ascontiguousarray(x.reshape(R, D).T))
    ident = np.eye(128, dtype=np.float32)
    # chunked-AG k order: (h_rel, rank, d) -> head h = 2*rank + h_rel
    k_perm = np.array(
        [2 * rank + h_rel for h_rel in range(2) for rank in range(NCORES)]
    )
    in_maps = []
    for c in range(NCORES):
        heads = [2 * c, 2 * c + 1]
        W1b_c = W1b[128 * c : 128 * (c + 1), :]  # [128 out-cols, D]
        # permute contraction (k) axis to (h_rel, rank, d) order
        W1b_c_perm = (
            W1b_c.reshape(128, H, HD)[:, k_perm, :].reshape(128, D)
        )
        in_maps.append(
            {
                "xT": xT,
                "w1aT": round_fp32r(
                    np.ascontiguousarray(W1a[128 * c : 128 * (c + 1), :].T)
                ),
                "w1bT": round_fp32r(np.ascontiguousarray(W1b_c_perm.T)),
                "trilAT": np.stack(
                    [round_fp32r(np.tril(mat2a[h]).T) for h in heads]
                ),
                "trilBT": np.stack(
                    [round_fp32r(np.tril(mat2b[h]).T) for h in heads]
                ),
                "wout": round_fp32r(w_out[heads].reshape(128, D)),
                "ident": ident,
            }
        )
    return in_maps


def kernel(x, W1a, W1b, mat2a, mat2b, w_out):
    x = np.asarray(x, dtype=np.float32)
    W1a = np.asarray(W1a, dtype=np.float32)
    W1b = np.asarray(W1b, dtype=np.float32)
    mat2a = np.asarray(mat2a, dtype=np.float32)
    mat2b = np.asarray(mat2b, dtype=np.float32)
    w_out = np.asarray(w_out, dtype=np.float32)

    if "nc" not in _NC_CACHE:
        _NC_CACHE["nc"] = build_nc()
    nc = _NC_CACHE["nc"]

    in_maps = prep_in_maps(x, W1a, W1b, mat2a, mat2b, w_out)
    res = run_bass_kernel_spmd(nc, in_maps, core_ids=list(range(NCORES)))
    out = np.zeros((R, D), np.float32)
    for c in range(NCORES):
        out += res.results[c]["out_part"]
    return out.reshape(B, L, D)


if __name__ == "__main__":
    rng = np.random.default_rng(0)
    inputs = {
        "x": rng.standard_normal((B, L, D), dtype=np.float32),
        "W1a": rng.standard_normal((D, D), dtype=np.float32) / D,
        "W1b": rng.standard_normal((D, D), dtype=np.float32) / D,
        "mat2a": rng.standard_normal((H, L, L), dtype=np.float32) / 32,
        "mat2b": rng.standard_normal((H, L, L), dtype=np.float32) / 32,
        "w_out": rng.standard_normal((H, HD, D), dtype=np.float32) / D,
    }
    out = kernel(**inputs)
    print("kernel ran, out shape", out.shape)


# revision 12
# speedup vs baseline: 1.1497x; 1.1497x over previous
"""Trainium2 Bass kernel for nn_Causal_Kron_Block_MLP.

Reference computation (B=4, L=2048, D=1024, H=16, HD=64):
    y1 = x @ W1a.T                                   # [B,L,D]
    z  = relu(einsum('hlm,bhmd->bhld', tril(mat2a), split_heads(y1)))
    y2 = merge_heads(z) @ W1b.T
    w  = einsum('hlm,bhmd->bhld', tril(mat2b), split_heads(y2))
    out = einsum('bhld,hde->ble', w, w_out)

Sharding: 8 cores, head-parallel — core c owns heads (2c, 2c+1).
Each core computes y1/z for its 2 heads over the full batch; an
AllGather (split in two chunks, overlapped with compute) exchanges z
(the only cross-head mixing point is W1b); each core then computes
the y2 columns for its heads, the tril_b stage, and a partial
head-sum of the output; the host sums the 8 partials.

Layouts (device, per core; r = global row index (b, l), R = 8192):
    y1mT/y2mT: per (h_rel, p, m-block) tiles [128 = m, 128 = (j, d)]
               built by PE-transposes fused with stages 1/3
    z_all0/1:  [512 = (rank, d), R] per h_rel chunk (AllGather out)
    wT_sb:     [128 = (h_rel, d), R]  (stage-4 output, reassembled)
    out_part:  [R, D] f32             (partial head-sum, DRAM)

All matmul operands are float32r (fp32 rounded to 11 mantissa bits;
~236 ns per [128x128]x[128x512] matmul vs ~920 ns for fp32). Inputs
are pre-rounded on the host; on-device intermediates are rounded by
PSUM->SBUF copies writing float32r. Causality: tril blocks entirely
above the diagonal are never loaded nor multiplied. DMAs are batched
via multi-dim access patterns to keep the Sync sequencer off the
critical path.
"""

import ml_dtypes
import numpy as np

import concourse.bass as bass
import concourse.mybir as mybir
import concourse.tile as tile
from concourse import bacc
from concourse.bass_utils import run_bass_kernel_spmd

B, L, D, H, HD = 4, 2048, 1024, 16, 64
NCORES = 8
R = B * L               # 8192 global rows
NB = 512                # moving free-dim per matmul
N_RB = R // NB          # 16 row-blocks of 512
N_KB_D = D // 128       # 8 k-blocks over model dim
N_MB = L // 128         # 16 m-blocks over seq per batch
N_LB = L // NB          # 4 l-blocks of 512 per batch
MB_G = 4                # tril m-blocks fetched per DMA
F32 = mybir.dt.float32
F32R = mybir.dt.float32r
BF16 = mybir.dt.bfloat16

_NC_CACHE = {}


def round_fp32r(a: np.ndarray) -> np.ndarray:
    """Round-to-nearest-even fp32 -> fp32r (11 mantissa bits kept)."""
    u = np.ascontiguousarray(a, dtype=np.float32).view(np.uint32)
    lsb = (u >> 12) & 1
    r = (u + np.uint32(0x7FF) + lsb) & np.uint32(0xFFFFF000)
    return r.view(np.float32)


def build_nc():
    """Build the single-NEFF SPMD kernel (same program on all 8 cores)."""
    nc = bacc.Bacc(None, target_bir_lowering=False)

    xT = nc.dram_tensor("xT", [D, R], F32R, kind="ExternalInput")
    w1aT = nc.dram_tensor("w1aT", [D, 128], F32R, kind="ExternalInput")
    # w1bT rows are host-permuted to the chunked-AllGather k order:
    # chunk h_rel, then (rank, d).
    w1bT = nc.dram_tensor("w1bT", [D, 128], BF16, kind="ExternalInput")
    trilAT = nc.dram_tensor("trilAT", [2, L, L], F32R, kind="ExternalInput")
    trilBT = nc.dram_tensor("trilBT", [2, L, L], F32R, kind="ExternalInput")
    wout = nc.dram_tensor("wout", [128, D], F32R, kind="ExternalInput")
    ident_in = nc.dram_tensor("ident", [128, 128], F32R, kind="ExternalInput")
    out_part = nc.dram_tensor("out_part", [R, D], F32, kind="ExternalOutput")

    with tile.TileContext(nc) as tc:
        with (
            tc.tile_pool(name="persist", bufs=1) as persist,
            tc.tile_pool(name="psmm", bufs=4, space="PSUM") as psmm,
            tc.tile_pool(name="pstr", bufs=3, space="PSUM") as pstr,
            tc.tile_pool(name="dram", bufs=1, space="DRAM") as dram,
        ):
            ident = persist.tile([128, 128], F32R, tag="ident")
            nc.sync.dma_start(out=ident[:], in_=ident_in[:])

            w1aT_sb = persist.tile([128, D], F32R, tag="w1aT")
            w1bT_sb = persist.tile([128, D], BF16, tag="w1bT")
            wout_sb = persist.tile([128, D], F32R, tag="wout")
            nc.sync.dma_start(
                out=w1aT_sb[:].rearrange("p (g n) -> p g n", g=N_KB_D),
                in_=w1aT[:].rearrange("(g p) n -> p g n", p=128),
            )
            nc.sync.dma_start(
                out=w1bT_sb[:].rearrange("p (g n) -> p g n", g=N_KB_D),
                in_=w1bT[:].rearrange("(g p) n -> p g n", p=128),
            )
            nc.sync.dma_start(out=wout_sb[:], in_=wout[:])

            # AllGather chunks: z_in[h_rel] [HD, R] -> z_all[h_rel] [8*HD, R]
            z_in = [
                dram.tile([HD, R], BF16, tag=f"z_in{h}", name=f"z_in{h}")
                for h in range(2)
            ]
            z_all = [
                dram.tile(
                    [NCORES * HD, R], BF16, tag=f"z_all{h}",
                    name=f"z_all{h}", addr_space="Shared",
                )
                for h in range(2)
            ]

            # -------- stage 1/3 + fused transpose glue ------------------
            def linear_stage(src_load, wT_sb_, dstmT, scope, rb_order=None):
                """dstmT tiles [128=m, 128=(j,d)] per (h_rel, p, mb) from
                out[128=(h_rel,d), r] = wT_sb_.T @ src, PE-transposed."""
                with nc.named_scope(scope):
                    for rb in (rb_order or range(N_RB)):
                        b, lc = rb // N_LB, rb % N_LB
                        p, j = b // 2, b % 2
                        ps = psmm.tile([128, NB], F32, tag="ps_mm")
                        src = src_load(rb)
                        for kb in range(N_KB_D):
                            nc.tensor.matmul(
                                ps[:],
                                wT_sb_[:, kb * 128 : (kb + 1) * 128],
                                src[:, kb * NB : (kb + 1) * NB],
                                start=(kb == 0),
                                stop=(kb == N_KB_D - 1),
                            )
                        yt = persist.tile(
                            [128, NB], F32R, tag="yt", bufs=4, name=f"yt_{scope}_{rb}"
                        )
                        nc.scalar.activation(
                            yt[:], ps[:], mybir.ActivationFunctionType.Copy
                        )
                        for h_rel in range(2):
                            for ml in range(NB // 128):
                                mb = lc * (NB // 128) + ml
                                pst = pstr.tile([128, HD], F32R, tag="ps_tr")
                                nc.tensor.transpose(
                                    pst[:],
                                    yt[
                                        h_rel * HD : (h_rel + 1) * HD,
                                        ml * 128 : (ml + 1) * 128,
                                    ],
                                    ident[
                                        h_rel * HD : (h_rel + 1) * HD,
                                        h_rel * HD : (h_rel + 1) * HD,
                                    ],
                                )
                                off = ((h_rel * 2 + p) * N_MB + mb) * 128
                                nc.vector.tensor_copy(
                                    dstmT[:, off + j * HD : off + (j + 1) * HD],
                                    pst[:],
                                )

            # -------- stages 2/4: out = y.T @ trilT (causal) ------------
            def tril_stage(trilT, srcmT, out_cb, tpool, scope, hl_list):
                # Full 512-wide m-block groups strictly below the diagonal,
                # then 4 diagonal m-blocks loaded without their zero prefix.
                with nc.named_scope(scope):
                    for h_rel, lb in hl_list:
                        n_mb = (lb + 1) * (NB // 128)
                        pss = [
                            psmm.tile(
                                [128, NB], F32, tag="ps_mm",
                                name=f"ps_{scope}_{h_rel}_{lb}_{pi}",
                            )
                            for pi in range(2)
                        ]
                        for mg in range(0, lb * MB_G, MB_G):
                            tblk = tpool.tile(
                                [128, MB_G * NB], F32R, tag="tril_blk",
                                name=f"tb_{scope}_{h_rel}_{lb}_{mg}",
                            )
                            nc.sync.dma_start(
                                out=tblk[:].rearrange(
                                    "p (g n) -> p g n", g=MB_G
                                ),
                                in_=trilT[
                                    h_rel,
                                    mg * 128 : (mg + MB_G) * 128,
                                    lb * NB : (lb + 1) * NB,
                                ].rearrange("(g p) n -> p g n", p=128),
                            )
                            for mi in range(MB_G):
                                mb = mg + mi
                                for p in range(2):
                                    off = ((h_rel * 2 + p) * N_MB + mb) * 128
                                    nc.tensor.matmul(
                                        pss[p][:],
                                        srcmT[:, off : off + 128],
                                        tblk[:, mi * NB : (mi + 1) * NB],
                                        start=(mb == 0),
                                        stop=False,
                                    )
                        # diagonal group: m-block lb*4+i has i*128 leading zeros
                        for i in range(MB_G):
                            mb = lb * MB_G + i
                            w = NB - i * 128
                            dblk = tpool.tile(
                                [128, NB], F32R, tag="diag_blk",
                                name=f"db_{scope}_{h_rel}_{lb}_{i}",
                            )
                            nc.sync.dma_start(
                                out=dblk[:, :w],
                                in_=trilT[
                                    h_rel,
                                    mb * 128 : (mb + 1) * 128,
                                    lb * NB + i * 128 : (lb + 1) * NB,
                                ],
                            )
                            for p in range(2):
                                off = ((h_rel * 2 + p) * N_MB + mb) * 128
                                nc.tensor.matmul(
                                    pss[p][:, i * 128 : NB],
                                    srcmT[:, off : off + 128],
                                    dblk[:, :w],
                                    start=(mb == 0),
                                    stop=(i == MB_G - 1),
                                )
                        for p in range(2):
                            out_cb(h_rel, p, lb, pss[p])

            # ================= phase A ==================================
            with (
                tc.tile_pool(name="xin", bufs=2) as xin,
                tc.tile_pool(name="trilA_p", bufs=3) as trilA_p,
                tc.tile_pool(name="mtA", bufs=1) as mtA,
            ):
                y1mT = mtA.tile([128, 2 * R], F32R, tag="y1mT")
                z_sb = mtA.tile([128, R], BF16, tag="z_sb")

                def x_load(rb):
                    xt = xin.tile([128, D // 128 * NB], F32R, tag="x_blk",
                                  name=f"x_{rb}")
                    nc.sync.dma_start(
                        out=xt[:].rearrange("p (g n) -> p g n", g=N_KB_D),
                        in_=xT[:, rb * NB : (rb + 1) * NB].rearrange(
                            "(g p) n -> p g n", p=128
                        ),
                    )
                    return xt

                linear_stage(x_load, w1aT_sb, y1mT, "s1")

                def z_out(h_rel, p, lb, ps):
                    base = (h_rel * 2 + p) * L
                    nc.scalar.activation(
                        z_sb[:, base + lb * NB : base + (lb + 1) * NB],
                        ps[:],
                        mybir.ActivationFunctionType.Relu,
                    )

                # per-h_rel: stage 2 chunk -> z_in DMAs -> AllGather, so the
                # first gather overlaps the second chunk's compute
                for h_rel in range(2):
                    tril_stage(trilAT, y1mT, z_out, trilA_p, f"s2h{h_rel}",
                               [(h_rel, lb) for lb in range(N_LB)])
                    with nc.named_scope(f"ag_in{h_rel}"):
                        for p in range(2):
                            for j in range(2):
                                b = 2 * p + j
                                nc.sync.dma_start(
                                    out=z_in[h_rel][:, b * L : (b + 1) * L],
                                    in_=z_sb[
                                        j * HD : (j + 1) * HD,
                                        (h_rel * 2 + p) * L
                                        : (h_rel * 2 + p + 1) * L,
                                    ],
                                )
                    nc.gpsimd.collective_compute(
                        "AllGather",
                        mybir.AluOpType.bypass,
                        replica_groups=[list(range(NCORES))],
                        ins=[z_in[h_rel].opt()],
                        outs=[z_all[h_rel].opt()],
                    )

            # ================= phase B ==================================
            with (
                tc.tile_pool(name="zin_p", bufs=2) as zin_p,
                tc.tile_pool(name="trilB_p", bufs=2) as trilB_p,
                tc.tile_pool(name="mtB", bufs=1) as mtB,
                tc.tile_pool(name="stg", bufs=3) as stg,
            ):
                y2mT = mtB.tile([128, 2 * R], F32R, tag="y2mT")
                wT_sb = mtB.tile([128, R], F32R, tag="wT_sb")

                def z_load(rb):
                    # 8 k-blocks: chunk h_rel = kb//4, rows (rank,d)
                    zt = zin_p.tile([128, D // 128 * NB], BF16, tag="z_blk",
                                    name=f"z_{rb}")
                    for h_rel in range(2):
                        nc.sync.dma_start(
                            out=zt[
                                :, h_rel * 4 * NB : (h_rel + 1) * 4 * NB
                            ].rearrange("p (g n) -> p g n", g=4),
                            in_=z_all[h_rel][
                                :, rb * NB : (rb + 1) * NB
                            ].rearrange("(g p) n -> p g n", p=128),
                        )
                    return zt

                linear_stage(z_load, w1bT_sb, y2mT, "s3",
                             rb_order=[b * N_LB + lc for lc in range(N_LB)
                                       for b in range(B)])

                def w_cb(h_rel, p, lb, ps):
                    st = stg.tile([128, NB], F32R, tag="w_stage",
                                  name=f"wst_{h_rel}_{p}_{lb}")
                    nc.scalar.activation(
                        st[:], ps[:], mybir.ActivationFunctionType.Copy
                    )
                    for j in range(2):
                        b = 2 * p + j
                        nc.sync.dma_start(
                            out=wT_sb[
                                h_rel * HD : (h_rel + 1) * HD,
                                b * L + lb * NB : b * L + (lb + 1) * NB,
                            ],
                            in_=st[j * HD : (j + 1) * HD, :],
                        )

                tril_stage(
                    trilBT, y2mT, w_cb, trilB_p, "s4",
                    [(h_rel, lb) for lb in range(N_LB) for h_rel in range(2)],
                )

                # stage 5: out_part rows = wT.T @ wout
                with nc.named_scope("s5"):
                    for rb in range(R // 128):
                        ost = stg.tile([128, D], F32, tag="out_stage",
                                       bufs=2, name=f"ost_{rb}")
                        for eh in range(2):
                            ps = psmm.tile([128, NB], F32, tag="ps_mm",
                                           name=f"ps5_{rb}_{eh}")
                            nc.tensor.matmul(
                                ps[:],
                                wT_sb[:, rb * 128 : (rb + 1) * 128],
                                wout_sb[:, eh * NB : (eh + 1) * NB],
                                start=True,
                                stop=True,
                            )
                            nc.scalar.activation(
                                ost[:, eh * NB : (eh + 1) * NB],
                                ps[:],
                                mybir.ActivationFunctionType.Copy,
                            )
                        nc.sync.dma_start(
                            out=out_part[rb * 128 : (rb + 1) * 128, :],
                            in_=ost[:],
                        )

    nc.finalize()
    return nc


def prep_in_maps(x, W1a, W1b, mat2a, mat2b, w_out):
    xT = round_fp32r(np.ascontiguousarray(x.reshape(R, D).T))
    ident = np.eye(128, dtype=np.float32)
    # chunked-AG k order: (h_rel, rank, d) -> head h = 2*rank + h_rel
    k_perm = np.array(
        [2 * rank + h_rel for h_rel in range(2) for rank in range(NCORES)]
    )
    in_maps = []
    for c in range(NCORES):
        heads = [2 * c, 2 * c + 1]
        W1b_c = W1b[128 * c : 128 * (c + 1), :]  # [128 out-cols, D]
        # permute contraction (k) axis to (h_rel, rank, d) order
        W1b_c_perm = (
            W1b_c.reshape(128, H, HD)[:, k_perm, :].reshape(128, D)
        )
        in_maps.append(
            {
                "xT": xT,
                "w1aT": round_fp32r(
                    np.ascontiguousarray(W1a[128 * c : 128 * (c + 1), :].T)
                ),
                "w1bT": np.ascontiguousarray(W1b_c_perm.T).astype(ml_dtypes.bfloat16),
                "trilAT": np.stack(
                    [round_fp32r(np.tril(mat2a[h]).T) for h in heads]
                ),
                "trilBT": np.stack(
                    [round_fp32r(np.tril(mat2b[h]).T) for h in heads]
                ),
                "wout": round_fp32r(w_out[heads].reshape(128, D)),
                "ident": ident,
            }
        )
    return in_maps


def kernel(x, W1a, W1b, mat2a, mat2b, w_out):
    x = np.asarray(x, dtype=np.float32)
    W1a = np.asarray(W1a, dtype=np.float32)
    W1b = np.asarray(W1b, dtype=np.float32)
    mat2a = np.asarray(mat2a, dtype=np.float32)
    mat2b = np.asarray(mat2b, dtype=np.float32)
    w_out = np.asarray(w_out, dtype=np.float32)

    if "nc" not in _NC_CACHE:
        _NC_CACHE["nc"] = build_nc()
    nc = _NC_CACHE["nc"]

    in_maps = prep_in_maps(x, W1a, W1b, mat2a, mat2b, w_out)
    res = run_bass_kernel_spmd(nc, in_maps, core_ids=list(range(NCORES)))
    out = np.zeros((R, D), np.float32)
    for c in range(NCORES):
        out += res.results[c]["out_part"]
    return out.reshape(B, L, D)


if __name__ == "__main__":
    rng = np.random.default_rng(0)
    inputs = {
        "x": rng.standard_normal((B, L, D), dtype=np.float32),
        "W1a": rng.standard_normal((D, D), dtype=np.float32) / D,
        "W1b": rng.standard_normal((D, D), dtype=np.float32) / D,
        "mat2a": rng.standard_normal((H, L, L), dtype=np.float32) / 32,
        "mat2b": rng.standard_normal((H, L, L), dtype=np.float32) / 32,
        "w_out": rng.standard_normal((H, HD, D), dtype=np.float32) / D,
    }
    out = kernel(**inputs)
    print("kernel ran, out shape", out.shape)


# revision 13
# speedup vs baseline: 1.1851x; 1.0307x over previous
"""Trainium2 Bass kernel for nn_Causal_Kron_Block_MLP.

Reference computation (B=4, L=2048, D=1024, H=16, HD=64):
    y1 = x @ W1a.T                                   # [B,L,D]
    z  = relu(einsum('hlm,bhmd->bhld', tril(mat2a), split_heads(y1)))
    y2 = merge_heads(z) @ W1b.T
    w  = einsum('hlm,bhmd->bhld', tril(mat2b), split_heads(y2))
    out = einsum('bhld,hde->ble', w, w_out)

Sharding: 8 cores, head-parallel — core c owns heads (2c, 2c+1).
Each core computes y1/z for its 2 heads over the full batch; an
AllGather (split in two chunks, overlapped with compute) exchanges z
(the only cross-head mixing point is W1b); each core then computes
the y2 columns for its heads, the tril_b stage, and a partial
head-sum of the output; the host sums the 8 partials.

Layouts (device, per core; r = global row index (b, l), R = 8192):
    y1mT/y2mT: per (h_rel, p, m-block) tiles [128 = m, 128 = (j, d)]
               built by PE-transposes fused with stages 1/3
    z_all0/1:  [512 = (rank, d), R] per h_rel chunk (AllGather out)
    wT_sb:     [128 = (h_rel, d), R]  (stage-4 output, reassembled)
    out_part:  [R, D] f32             (partial head-sum, DRAM)

All matmul operands are float32r (fp32 rounded to 11 mantissa bits;
~236 ns per [128x128]x[128x512] matmul vs ~920 ns for fp32). Inputs
are pre-rounded on the host; on-device intermediates are rounded by
PSUM->SBUF copies writing float32r. Causality: tril blocks entirely
above the diagonal are never loaded nor multiplied. DMAs are batched
via multi-dim access patterns to keep the Sync sequencer off the
critical path.
"""

import numpy as np

import concourse.bass as bass
import concourse.mybir as mybir
import concourse.tile as tile
from concourse import bacc
from concourse.bass_utils import run_bass_kernel_spmd

B, L, D, H, HD = 4, 2048, 1024, 16, 64
NCORES = 8
R = B * L               # 8192 global rows
NB = 512                # moving free-dim per matmul
N_RB = R // NB          # 16 row-blocks of 512
N_KB_D = D // 128       # 8 k-blocks over model dim
N_MB = L // 128         # 16 m-blocks over seq per batch
N_LB = L // NB          # 4 l-blocks of 512 per batch
MB_G = 4                # tril m-blocks fetched per DMA
F32 = mybir.dt.float32
F32R = mybir.dt.float32r
FP16 = mybir.dt.float16

_NC_CACHE = {}


def round_fp32r(a: np.ndarray) -> np.ndarray:
    """Round-to-nearest-even fp32 -> fp32r (11 mantissa bits kept)."""
    u = np.ascontiguousarray(a, dtype=np.float32).view(np.uint32)
    lsb = (u >> 12) & 1
    r = (u + np.uint32(0x7FF) + lsb) & np.uint32(0xFFFFF000)
    return r.view(np.float32)


def build_nc():
    """Build the single-NEFF SPMD kernel (same program on all 8 cores)."""
    nc = bacc.Bacc(None, target_bir_lowering=False)

    xT = nc.dram_tensor("xT", [D, R], F32R, kind="ExternalInput")
    w1aT = nc.dram_tensor("w1aT", [D, 128], F32R, kind="ExternalInput")
    # w1bT rows are host-permuted to the chunked-AllGather k order:
    # chunk h_rel, then (rank, d).
    w1bT = nc.dram_tensor("w1bT", [D, 128], FP16, kind="ExternalInput")
    trilAT = nc.dram_tensor("trilAT", [2, L, L], F32R, kind="ExternalInput")
    trilBT = nc.dram_tensor("trilBT", [2, L, L], F32R, kind="ExternalInput")
    wout = nc.dram_tensor("wout", [128, D], F32R, kind="ExternalInput")
    ident_in = nc.dram_tensor("ident", [128, 128], F32R, kind="ExternalInput")
    out_part = nc.dram_tensor("out_part", [R, D], F32, kind="ExternalOutput")

    with tile.TileContext(nc) as tc:
        with (
            tc.tile_pool(name="persist", bufs=1) as persist,
            tc.tile_pool(name="psmm", bufs=4, space="PSUM") as psmm,
            tc.tile_pool(name="pstr", bufs=3, space="PSUM") as pstr,
            tc.tile_pool(name="dram", bufs=1, space="DRAM") as dram,
        ):
            ident = persist.tile([128, 128], F32R, tag="ident")
            nc.sync.dma_start(out=ident[:], in_=ident_in[:])

            w1aT_sb = persist.tile([128, D], F32R, tag="w1aT")
            w1bT_sb = persist.tile([128, D], FP16, tag="w1bT")
            wout_sb = persist.tile([128, D], F32R, tag="wout")
            nc.sync.dma_start(
                out=w1aT_sb[:].rearrange("p (g n) -> p g n", g=N_KB_D),
                in_=w1aT[:].rearrange("(g p) n -> p g n", p=128),
            )
            nc.sync.dma_start(
                out=w1bT_sb[:].rearrange("p (g n) -> p g n", g=N_KB_D),
                in_=w1bT[:].rearrange("(g p) n -> p g n", p=128),
            )
            nc.sync.dma_start(out=wout_sb[:], in_=wout[:])

            # AllGather chunks: z_in[h_rel] [HD, R] -> z_all[h_rel] [8*HD, R]
            z_in = [
                dram.tile([HD, R], FP16, tag=f"z_in{h}", name=f"z_in{h}")
                for h in range(2)
            ]
            z_all = [
                dram.tile(
                    [NCORES * HD, R], FP16, tag=f"z_all{h}",
                    name=f"z_all{h}", addr_space="Shared",
                )
                for h in range(2)
            ]

            # -------- stage 1/3 + fused transpose glue ------------------
            def linear_stage(src_load, wT_sb_, dstmT, scope, rb_order=None):
                """dstmT tiles [128=m, 128=(j,d)] per (h_rel, p, mb) from
                out[128=(h_rel,d), r] = wT_sb_.T @ src, PE-transposed."""
                with nc.named_scope(scope):
                    for rb in (rb_order or range(N_RB)):
                        b, lc = rb // N_LB, rb % N_LB
                        p, j = b // 2, b % 2
                        ps = psmm.tile([128, NB], F32, tag="ps_mm")
                        src = src_load(rb)
                        for kb in range(N_KB_D):
                            nc.tensor.matmul(
                                ps[:],
                                wT_sb_[:, kb * 128 : (kb + 1) * 128],
                                src[:, kb * NB : (kb + 1) * NB],
                                start=(kb == 0),
                                stop=(kb == N_KB_D - 1),
                            )
                        yt = persist.tile(
                            [128, NB], F32R, tag="yt", bufs=4, name=f"yt_{scope}_{rb}"
                        )
                        nc.scalar.activation(
                            yt[:], ps[:], mybir.ActivationFunctionType.Copy
                        )
                        for h_rel in range(2):
                            for ml in range(NB // 128):
                                mb = lc * (NB // 128) + ml
                                pst = pstr.tile([128, HD], F32R, tag="ps_tr")
                                nc.tensor.transpose(
                                    pst[:],
                                    yt[
                                        h_rel * HD : (h_rel + 1) * HD,
                                        ml * 128 : (ml + 1) * 128,
                                    ],
                                    ident[
                                        h_rel * HD : (h_rel + 1) * HD,
                                        h_rel * HD : (h_rel + 1) * HD,
                                    ],
                                )
                                off = ((h_rel * 2 + p) * N_MB + mb) * 128
                                nc.vector.tensor_copy(
                                    dstmT[:, off + j * HD : off + (j + 1) * HD],
                                    pst[:],
                                )

            # -------- stages 2/4: out = y.T @ trilT (causal) ------------
            def tril_stage(trilT, srcmT, out_cb, tpool, scope, hl_list):
                # Full 512-wide m-block groups strictly below the diagonal,
                # then 4 diagonal m-blocks loaded without their zero prefix.
                with nc.named_scope(scope):
                    for h_rel, lb in hl_list:
                        n_mb = (lb + 1) * (NB // 128)
                        pss = [
                            psmm.tile(
                                [128, NB], F32, tag="ps_mm",
                                name=f"ps_{scope}_{h_rel}_{lb}_{pi}",
                            )
                            for pi in range(2)
                        ]
                        for mg in range(0, lb * MB_G, MB_G):
                            tblk = tpool.tile(
                                [128, MB_G * NB], F32R, tag="tril_blk",
                                name=f"tb_{scope}_{h_rel}_{lb}_{mg}",
                            )
                            nc.sync.dma_start(
                                out=tblk[:].rearrange(
                                    "p (g n) -> p g n", g=MB_G
                                ),
                                in_=trilT[
                                    h_rel,
                                    mg * 128 : (mg + MB_G) * 128,
                                    lb * NB : (lb + 1) * NB,
                                ].rearrange("(g p) n -> p g n", p=128),
                            )
                            for mi in range(MB_G):
                                mb = mg + mi
                                for p in range(2):
                                    off = ((h_rel * 2 + p) * N_MB + mb) * 128
                                    nc.tensor.matmul(
                                        pss[p][:],
                                        srcmT[:, off : off + 128],
                                        tblk[:, mi * NB : (mi + 1) * NB],
                                        start=(mb == 0),
                                        stop=False,
                                    )
                        # diagonal group: m-block lb*4+i has i*128 leading zeros
                        for i in range(MB_G):
                            mb = lb * MB_G + i
                            w = NB - i * 128
                            dblk = tpool.tile(
                                [128, NB], F32R, tag="diag_blk",
                                name=f"db_{scope}_{h_rel}_{lb}_{i}",
                            )
                            nc.sync.dma_start(
                                out=dblk[:, :w],
                                in_=trilT[
                                    h_rel,
                                    mb * 128 : (mb + 1) * 128,
                                    lb * NB + i * 128 : (lb + 1) * NB,
                                ],
                            )
                            for p in range(2):
                                off = ((h_rel * 2 + p) * N_MB + mb) * 128
                                nc.tensor.matmul(
                                    pss[p][:, i * 128 : NB],
                                    srcmT[:, off : off + 128],
                                    dblk[:, :w],
                                    start=(mb == 0),
                                    stop=(i == MB_G - 1),
                                )
                        for p in range(2):
                            out_cb(h_rel, p, lb, pss[p])

            # ================= phase A ==================================
            with (
                tc.tile_pool(name="xin", bufs=2) as xin,
                tc.tile_pool(name="trilA_p", bufs=3) as trilA_p,
                tc.tile_pool(name="mtA", bufs=1) as mtA,
            ):
                y1mT = mtA.tile([128, 2 * R], F32R, tag="y1mT")
                z_sb = mtA.tile([128, R], FP16, tag="z_sb")

                def x_load(rb):
                    xt = xin.tile([128, D // 128 * NB], F32R, tag="x_blk",
                                  name=f"x_{rb}")
                    nc.sync.dma_start(
                        out=xt[:].rearrange("p (g n) -> p g n", g=N_KB_D),
                        in_=xT[:, rb * NB : (rb + 1) * NB].rearrange(
                            "(g p) n -> p g n", p=128
                        ),
                    )
                    return xt

                linear_stage(x_load, w1aT_sb, y1mT, "s1")

                def z_out(h_rel, p, lb, ps):
                    base = (h_rel * 2 + p) * L
                    nc.scalar.activation(
                        z_sb[:, base + lb * NB : base + (lb + 1) * NB],
                        ps[:],
                        mybir.ActivationFunctionType.Relu,
                    )

                # per-h_rel: stage 2 chunk -> z_in DMAs -> AllGather, so the
                # first gather overlaps the second chunk's compute
                for h_rel in range(2):
                    tril_stage(trilAT, y1mT, z_out, trilA_p, f"s2h{h_rel}",
                               [(h_rel, lb) for lb in range(N_LB)])
                    with nc.named_scope(f"ag_in{h_rel}"):
                        for p in range(2):
                            for j in range(2):
                                b = 2 * p + j
                                nc.sync.dma_start(
                                    out=z_in[h_rel][:, b * L : (b + 1) * L],
                                    in_=z_sb[
                                        j * HD : (j + 1) * HD,
                                        (h_rel * 2 + p) * L
                                        : (h_rel * 2 + p + 1) * L,
                                    ],
                                )
                    nc.gpsimd.collective_compute(
                        "AllGather",
                        mybir.AluOpType.bypass,
                        replica_groups=[list(range(NCORES))],
                        ins=[z_in[h_rel].opt()],
                        outs=[z_all[h_rel].opt()],
                    )

            # ================= phase B ==================================
            with (
                tc.tile_pool(name="zin_p", bufs=2) as zin_p,
                tc.tile_pool(name="trilB_p", bufs=2) as trilB_p,
                tc.tile_pool(name="mtB", bufs=1) as mtB,
                tc.tile_pool(name="stg", bufs=3) as stg,
            ):
                y2mT = mtB.tile([128, 2 * R], F32R, tag="y2mT")
                wT_sb = mtB.tile([128, R], F32R, tag="wT_sb")

                def z_load(rb):
                    # 8 k-blocks: chunk h_rel = kb//4, rows (rank,d)
                    zt = zin_p.tile([128, D // 128 * NB], FP16, tag="z_blk",
                                    name=f"z_{rb}")
                    for h_rel in range(2):
                        nc.sync.dma_start(
                            out=zt[
                                :, h_rel * 4 * NB : (h_rel + 1) * 4 * NB
                            ].rearrange("p (g n) -> p g n", g=4),
                            in_=z_all[h_rel][
                                :, rb * NB : (rb + 1) * NB
                            ].rearrange("(g p) n -> p g n", p=128),
                        )
                    return zt

                linear_stage(z_load, w1bT_sb, y2mT, "s3",
                             rb_order=[b * N_LB + lc for lc in range(N_LB)
                                       for b in range(B)])

                def w_cb(h_rel, p, lb, ps):
                    st = stg.tile([128, NB], F32R, tag="w_stage",
                                  name=f"wst_{h_rel}_{p}_{lb}")
                    nc.scalar.activation(
                        st[:], ps[:], mybir.ActivationFunctionType.Copy
                    )
                    for j in range(2):
                        b = 2 * p + j
                        nc.sync.dma_start(
                            out=wT_sb[
                                h_rel * HD : (h_rel + 1) * HD,
                                b * L + lb * NB : b * L + (lb + 1) * NB,
                            ],
                            in_=st[j * HD : (j + 1) * HD, :],
                        )

                tril_stage(
                    trilBT, y2mT, w_cb, trilB_p, "s4",
                    [(h_rel, lb) for lb in range(N_LB) for h_rel in range(2)],
                )

                # stage 5: out_part rows = wT.T @ wout
                with nc.named_scope("s5"):
                    for rb in range(R // 128):
                        ost = stg.tile([128, D], F32, tag="out_stage",
                                       bufs=2, name=f"ost_{rb}")
                        for eh in range(2):
                            ps = psmm.tile([128, NB], F32, tag="ps_mm",
                                           name=f"ps5_{rb}_{eh}")
                            nc.tensor.matmul(
                                ps[:],
                                wT_sb[:, rb * 128 : (rb + 1) * 128],
                                wout_sb[:, eh * NB : (eh + 1) * NB],
                                start=True,
                                stop=True,
                            )
                            nc.scalar.activation(
                                ost[:, eh * NB : (eh + 1) * NB],
                                ps[:],
                                mybir.ActivationFunctionType.Copy,
                            )
                        nc.sync.dma_start(
                            out=out_part[rb * 128 : (rb + 1) * 128, :],
                            in_=ost[:],
                        )

    nc.finalize()
    return nc


def prep_in_maps(x, W1a, W1b, mat2a, mat2b, w_out):
    xT = round_fp32r(np.ascontiguousarray(x.reshape(R, D).T))
    ident = np.eye(128, dtype=np.float32)
    # chunked-AG k order: (h_rel, rank, d) -> head h = 2*rank + h_rel
    k_perm = np.array(
        [2 * rank + h_rel for h_rel in range(2) for rank in range(NCORES)]
    )
    in_maps = []
    for c in range(NCORES):
        heads = [2 * c, 2 * c + 1]
        W1b_c = W1b[128 * c : 128 * (c + 1), :]  # [128 out-cols, D]
        # permute contraction (k) axis to (h_rel, rank, d) order
        W1b_c_perm = (
            W1b_c.reshape(128, H, HD)[:, k_perm, :].reshape(128, D)
        )
        in_maps.append(
            {
                "xT": xT,
                "w1aT": round_fp32r(
                    np.ascontiguousarray(W1a[128 * c : 128 * (c + 1), :].T)
                ),
                "w1bT": np.ascontiguousarray(W1b_c_perm.T).astype(np.float16),
                "trilAT": np.stack(
                    [round_fp32r(np.tril(mat2a[h]).T) for h in heads]
                ),
                "trilBT": np.stack(
                    [round_fp32r(np.tril(mat2b[h]).T) for h in heads]
                ),
                "wout": round_fp32r(w_out[heads].reshape(128, D)),
                "ident": ident,
            }
        )
    return in_maps


def kernel(x, W1a, W1b, mat2a, mat2b, w_out):
    x = np.asarray(x, dtype=np.float32)
    W1a = np.asarray(W1a, dtype=np.float32)
    W1b = np.asarray(W1b, dtype=np.float32)
    mat2a = np.asarray(mat2a, dtype=np.float32)
    mat2b = np.asarray(mat2b, dtype=np.float32)
    w_out = np.asarray(w_out, dtype=np.float32)

    if "nc" not in _NC_CACHE:
        _NC_CACHE["nc"] = build_nc()
    nc = _NC_CACHE["nc"]

    in_maps = prep_in_maps(x, W1a, W1b, mat2a, mat2b, w_out)
    res = run_bass_kernel_spmd(nc, in_maps, core_ids=list(range(NCORES)))
    out = np.zeros((R, D), np.float32)
    for c in range(NCORES):
        out += res.results[c]["out_part"]
    return out.reshape(B, L, D)


if __name__ == "__main__":
    rng = np.random.default_rng(0)
    inputs = {
        "x": rng.standard_normal((B, L, D), dtype=np.float32),
        "W1a": rng.standard_normal((D, D), dtype=np.float32) / D,
        "W1b": rng.standard_normal((D, D), dtype=np.float32) / D,
        "mat2a": rng.standard_normal((H, L, L), dtype=np.float32) / 32,
        "mat2b": rng.standard_normal((H, L, L), dtype=np.float32) / 32,
        "w_out": rng.standard_normal((H, HD, D), dtype=np.float32) / D,
    }
    out = kernel(**inputs)
    print("kernel ran, out shape", out.shape)


# revision 17
# speedup vs baseline: 1.8290x; 1.5434x over previous
"""Trainium2 Bass kernel for nn_Causal_Kron_Block_MLP.

Reference computation (B=4, L=2048, D=1024, H=16, HD=64):
    y1 = x @ W1a.T                                   # [B,L,D]
    z  = relu(einsum('hlm,bhmd->bhld', tril(mat2a), split_heads(y1)))
    y2 = merge_heads(z) @ W1b.T
    w  = einsum('hlm,bhmd->bhld', tril(mat2b), split_heads(y2))
    out = einsum('bhld,hde->ble', w, w_out)

Sharding: 8 cores, head-parallel — core c owns heads (2c, 2c+1).
Each core computes y1/z for its 2 heads over the full batch; an
AllGather (split in two chunks, overlapped with compute) exchanges z
(the only cross-head mixing point is W1b); each core then computes
the y2 columns for its heads, the tril_b stage, and a partial
head-sum of the output; the host sums the 8 partials.

Layouts (device, per core; r = global row index (b, l), R = 8192):
    y1mT/y2mT: per (h_rel, p, m-block) tiles [128 = m, 128 = (j, d)]
               built by PE-transposes fused with stages 1/3
    z_all0/1:  [512 = (rank, d), R] per h_rel chunk (AllGather out)
    wT_sb:     [128 = (h_rel, d), R]  (stage-4 output, reassembled)
    out_part:  [R, D] fp16, scaled by 1024 (values ~1e-5 would be
               fp16-subnormal unscaled); the host sums in f32 and
               rescales.

All matmuls run in fp16 (1 PE cycle/row, 10 mantissa bits; inputs
pre-cast on the host, intermediates rounded by the PSUM->SBUF copies)
with f32 PSUM accumulation; measured end-to-end relative error vs the
f32 reference is ~1e-3. Causality: tril blocks entirely above the
diagonal are never loaded nor multiplied; diagonal blocks skip their
zero prefix. DMAs are batched via multi-dim access patterns.
"""

import numpy as np

import concourse.bass as bass
import concourse.mybir as mybir
import concourse.tile as tile
from concourse import bacc
from concourse.bass_utils import run_bass_kernel_spmd

B, L, D, H, HD = 4, 2048, 1024, 16, 64
NCORES = 8
R = B * L               # 8192 global rows
NB = 512                # moving free-dim per matmul
N_RB = R // NB          # 16 row-blocks of 512
N_KB_D = D // 128       # 8 k-blocks over model dim
N_MB = L // 128         # 16 m-blocks over seq per batch
N_LB = L // NB          # 4 l-blocks of 512 per batch
MB_G = 4                # tril m-blocks fetched per DMA
OUT_SCALE = 1024.0
F32 = mybir.dt.float32
FP16 = mybir.dt.float16

_NC_CACHE = {}


def build_nc():
    """Build the single-NEFF SPMD kernel (same program on all 8 cores)."""
    nc = bacc.Bacc(None, target_bir_lowering=False)

    xT = nc.dram_tensor("xT", [D, R], FP16, kind="ExternalInput")
    w1aT = nc.dram_tensor("w1aT", [D, 128], FP16, kind="ExternalInput")
    # w1bT rows are host-permuted to the chunked-AllGather k order:
    # chunk h_rel, then (rank, d).
    w1bT = nc.dram_tensor("w1bT", [D, 128], FP16, kind="ExternalInput")
    trilAT = nc.dram_tensor("trilAT", [2, L, L], FP16, kind="ExternalInput")
    trilBT = nc.dram_tensor("trilBT", [2, L, L], FP16, kind="ExternalInput")
    wout = nc.dram_tensor("wout", [128, D], FP16, kind="ExternalInput")
    ident_in = nc.dram_tensor("ident", [128, 128], FP16, kind="ExternalInput")
    out_part = nc.dram_tensor("out_part", [R, D], FP16, kind="ExternalOutput")

    with tile.TileContext(nc) as tc:
        with (
            tc.tile_pool(name="persist", bufs=1) as persist,
            tc.tile_pool(name="stg0", bufs=3) as stg0,
            tc.tile_pool(name="psmm", bufs=5, space="PSUM") as psmm,
            tc.tile_pool(name="pstr", bufs=3, space="PSUM") as pstr,
            tc.tile_pool(name="dram", bufs=1, space="DRAM") as dram,
        ):
            ident = persist.tile([128, 128], FP16, tag="ident")
            nc.sync.dma_start(out=ident[:], in_=ident_in[:])

            w1aT_sb = persist.tile([128, D], FP16, tag="w1aT")
            w1bT_sb = persist.tile([128, D], FP16, tag="w1bT")
            wout_sb = persist.tile([128, D], FP16, tag="wout")
            nc.sync.dma_start(
                out=w1aT_sb[:].rearrange("p (g n) -> p g n", g=N_KB_D),
                in_=w1aT[:].rearrange("(g p) n -> p g n", p=128),
            )
            nc.sync.dma_start(
                out=w1bT_sb[:].rearrange("p (g n) -> p g n", g=N_KB_D),
                in_=w1bT[:].rearrange("(g p) n -> p g n", p=128),
            )
            nc.sync.dma_start(out=wout_sb[:], in_=wout[:])

            # AllGather chunks: z_in[h_rel] [HD, R] -> z_all[h_rel] [8*HD, R]
            z_in = [
                dram.tile([HD, R], FP16, tag=f"z_in{h}", name=f"z_in{h}")
                for h in range(2)
            ]
            z_all = [
                dram.tile(
                    [NCORES * HD, R], FP16, tag=f"z_all{h}",
                    name=f"z_all{h}", addr_space="Shared",
                )
                for h in range(2)
            ]

            # -------- stage 1/3 + fused transpose glue ------------------
            def linear_stage(src_loads, wT_sb_, dstmT, scope, rb_order=None,
                             split_chains=False):
                """dstmT tiles [128=m, 128=(j,d)] per (h_rel, p, mb) from
                out[128=(h_rel,d), r] = wT_sb_.T @ src, PE-transposed.
                src_loads(rb) -> list of (tile, kb_lo, kb_hi).
                split_chains: one PSUM chain per source group (so a group's
                matmuls don't wait on later groups' inputs), summed after."""
                with nc.named_scope(scope):
                    for rb in (rb_order or range(N_RB)):
                        b, lc = rb // N_LB, rb % N_LB
                        p, j = b // 2, b % 2
                        groups = src_loads(rb)
                        yt = persist.tile(
                            [128, NB], FP16, tag="yt", bufs=6,
                            name=f"yt_{scope}_{rb}",
                        )
                        if not split_chains or len(groups) == 1:
                            ps = psmm.tile([128, NB], F32, tag="ps_mm")
                            for src, kb_lo, kb_hi in groups:
                                for kb in range(kb_lo, kb_hi):
                                    nc.tensor.matmul(
                                        ps[:],
                                        wT_sb_[:, kb * 128 : (kb + 1) * 128],
                                        src[
                                            :,
                                            (kb - kb_lo) * NB
                                            : (kb - kb_lo + 1) * NB,
                                        ],
                                        start=(kb == 0),
                                        stop=(kb == N_KB_D - 1),
                                    )
                            nc.scalar.activation(
                                yt[:], ps[:], mybir.ActivationFunctionType.Copy
                            )
                        else:
                            ps_list = []
                            for src, kb_lo, kb_hi in groups:
                                ps = psmm.tile(
                                    [128, NB], F32, tag="ps_mm",
                                    name=f"ps_{scope}_{rb}_{kb_lo}",
                                )
                                for kb in range(kb_lo, kb_hi):
                                    nc.tensor.matmul(
                                        ps[:],
                                        wT_sb_[:, kb * 128 : (kb + 1) * 128],
                                        src[
                                            :,
                                            (kb - kb_lo) * NB
                                            : (kb - kb_lo + 1) * NB,
                                        ],
                                        start=(kb == kb_lo),
                                        stop=(kb == kb_hi - 1),
                                    )
                                ps_list.append(ps)
                            tmp = stg0.tile([128, NB], F32, tag="ysum",
                                            name=f"ys_{scope}_{rb}")
                            nc.scalar.activation(
                                tmp[:], ps_list[0][:],
                                mybir.ActivationFunctionType.Copy,
                            )
                            nc.vector.tensor_tensor(
                                yt[:], tmp[:], ps_list[1][:],
                                mybir.AluOpType.add,
                            )
                        # one 128-wide transpose covers both h_rel halves
                        for ml in range(NB // 128):
                            mb = lc * (NB // 128) + ml
                            pst = pstr.tile([128, 128], FP16, tag="ps_tr")
                            nc.tensor.transpose(
                                pst[:],
                                yt[:, ml * 128 : (ml + 1) * 128],
                                ident[:],
                            )
                            for h_rel in range(2):
                                off = ((h_rel * 2 + p) * N_MB + mb) * 128
                                nc.vector.tensor_copy(
                                    dstmT[:, off + j * HD : off + (j + 1) * HD],
                                    pst[:, h_rel * HD : (h_rel + 1) * HD],
                                )

            # -------- stages 2/4: out = y.T @ trilT (causal) ------------
            def tril_stage(trilT, srcmT, out_cb, tpool, scope, hl_list):
                # Full 512-wide m-block groups strictly below the diagonal,
                # then 4 diagonal m-blocks loaded without their zero prefix.
                with nc.named_scope(scope):
                    for h_rel, lb in hl_list:
                        n_mb = (lb + 1) * (NB // 128)
                        pss = [
                            psmm.tile(
                                [128, NB], F32, tag="ps_mm",
                                name=f"ps_{scope}_{h_rel}_{lb}_{pi}",
                            )
                            for pi in range(2)
                        ]
                        for mg in range(0, lb * MB_G, MB_G):
                            tblk = tpool.tile(
                                [128, MB_G * NB], FP16, tag="tril_blk",
                                name=f"tb_{scope}_{h_rel}_{lb}_{mg}",
                            )
                            nc.sync.dma_start(
                                out=tblk[:].rearrange(
                                    "p (g n) -> p g n", g=MB_G
                                ),
                                in_=trilT[
                                    h_rel,
                                    mg * 128 : (mg + MB_G) * 128,
                                    lb * NB : (lb + 1) * NB,
                                ].rearrange("(g p) n -> p g n", p=128),
                            )
                            for mi in range(MB_G):
                                mb = mg + mi
                                for p in range(2):
                                    off = ((h_rel * 2 + p) * N_MB + mb) * 128
                                    nc.tensor.matmul(
                                        pss[p][:],
                                        srcmT[:, off : off + 128],
                                        tblk[:, mi * NB : (mi + 1) * NB],
                                        start=(mb == 0),
                                        stop=False,
                                    )
                        # diagonal group: m-block lb*4+i has i*128 leading zeros
                        for i in range(MB_G):
                            mb = lb * MB_G + i
                            w = NB - i * 128
                            dblk = tpool.tile(
                                [128, NB], FP16, tag="diag_blk",
                                name=f"db_{scope}_{h_rel}_{lb}_{i}",
                            )
                            nc.sync.dma_start(
                                out=dblk[:, :w],
                                in_=trilT[
                                    h_rel,
                                    mb * 128 : (mb + 1) * 128,
                                    lb * NB + i * 128 : (lb + 1) * NB,
                                ],
                            )
                            for p in range(2):
                                off = ((h_rel * 2 + p) * N_MB + mb) * 128
                                nc.tensor.matmul(
                                    pss[p][:, i * 128 : NB],
                                    srcmT[:, off : off + 128],
                                    dblk[:, :w],
                                    start=(mb == 0),
                                    stop=(i == MB_G - 1),
                                )
                        for p in range(2):
                            out_cb(h_rel, p, lb, pss[p])

            # ================= phase A ==================================
            with (
                tc.tile_pool(name="xin", bufs=3) as xin,
                tc.tile_pool(name="trilA_p", bufs=5) as trilA_p,
                tc.tile_pool(name="mtA", bufs=1) as mtA,
            ):
                y1mT = mtA.tile([128, 2 * R], FP16, tag="y1mT")
                z_sb = mtA.tile([128, R], FP16, tag="z_sb")

                def x_load(rb):
                    xt = xin.tile([128, N_KB_D * NB], FP16, tag="x_blk",
                                  name=f"x_{rb}")
                    nc.sync.dma_start(
                        out=xt[:].rearrange("p (g n) -> p g n", g=N_KB_D),
                        in_=xT[:, rb * NB : (rb + 1) * NB].rearrange(
                            "(g p) n -> p g n", p=128
                        ),
                    )
                    return [(xt, 0, N_KB_D)]

                linear_stage(x_load, w1aT_sb, y1mT, "s1")

                def z_out(h_rel, p, lb, ps):
                    base = (h_rel * 2 + p) * L
                    nc.scalar.activation(
                        z_sb[:, base + lb * NB : base + (lb + 1) * NB],
                        ps[:],
                        mybir.ActivationFunctionType.Relu,
                    )

                # per-h_rel: stage 2 chunk -> z_in DMAs -> AllGather, so the
                # first gather overlaps the second chunk's compute
                for h_rel in range(2):
                    tril_stage(trilAT, y1mT, z_out, trilA_p, f"s2h{h_rel}",
                               [(h_rel, lb) for lb in range(N_LB)])
                    with nc.named_scope(f"ag_in{h_rel}"):
                        for p in range(2):
                            for j in range(2):
                                b = 2 * p + j
                                nc.sync.dma_start(
                                    out=z_in[h_rel][:, b * L : (b + 1) * L],
                                    in_=z_sb[
                                        j * HD : (j + 1) * HD,
                                        (h_rel * 2 + p) * L
                                        : (h_rel * 2 + p + 1) * L,
                                    ],
                                )
                    nc.gpsimd.collective_compute(
                        "AllGather",
                        mybir.AluOpType.bypass,
                        replica_groups=[list(range(NCORES))],
                        ins=[z_in[h_rel].opt()],
                        outs=[z_all[h_rel].opt()],
                    )

            # ================= phase B ==================================
            with (
                tc.tile_pool(name="zin_p", bufs=3) as zin_p,
                tc.tile_pool(name="trilB_p", bufs=5) as trilB_p,
                tc.tile_pool(name="mtB", bufs=1) as mtB,
                tc.tile_pool(name="stg", bufs=3) as stg,
            ):
                y2mT = mtB.tile([128, 2 * R], FP16, tag="y2mT")
                wT_sb = mtB.tile([128, R], FP16, tag="wT_sb")

                def z_load(rb):
                    # separate tiles per AG chunk so chunk-0 matmuls don't
                    # wait for the second AllGather
                    out = []
                    for h_rel in range(2):
                        zt = zin_p.tile(
                            [128, 4 * NB], FP16, tag=f"z_blk{h_rel}",
                            name=f"z_{h_rel}_{rb}",
                        )
                        nc.sync.dma_start(
                            out=zt[:].rearrange("p (g n) -> p g n", g=4),
                            in_=z_all[h_rel][
                                :, rb * NB : (rb + 1) * NB
                            ].rearrange("(g p) n -> p g n", p=128),
                        )
                        out.append((zt, h_rel * 4, h_rel * 4 + 4))
                    return out

                linear_stage(
                    z_load, w1bT_sb, y2mT, "s3",
                    rb_order=[b * N_LB + lc for lc in range(N_LB)
                              for b in range(B)],
                    split_chains=True,
                )

                def w_cb(h_rel, p, lb, ps):
                    st = stg.tile([128, NB], FP16, tag="w_stage",
                                  name=f"wst_{h_rel}_{p}_{lb}")
                    nc.scalar.activation(
                        st[:], ps[:], mybir.ActivationFunctionType.Copy
                    )
                    for j in range(2):
                        b = 2 * p + j
                        nc.sync.dma_start(
                            out=wT_sb[
                                h_rel * HD : (h_rel + 1) * HD,
                                b * L + lb * NB : b * L + (lb + 1) * NB,
                            ],
                            in_=st[j * HD : (j + 1) * HD, :],
                        )

                tril_stage(
                    trilBT, y2mT, w_cb, trilB_p, "s4",
                    [(h_rel, lb) for lb in range(N_LB) for h_rel in range(2)],
                )

                # stage 5: out_part rows = (wT.T @ wout) * OUT_SCALE
                with nc.named_scope("s5"):
                    for rb in range(R // 128):
                        ost = stg.tile([128, D], FP16, tag="out_stage",
                                       bufs=3, name=f"ost_{rb}")
                        for eh in range(2):
                            ps = psmm.tile([128, NB], F32, tag="ps_mm",
                                           name=f"ps5_{rb}_{eh}")
                            nc.tensor.matmul(
                                ps[:],
                                wT_sb[:, rb * 128 : (rb + 1) * 128],
                                wout_sb[:, eh * NB : (eh + 1) * NB],
                                start=True,
                                stop=True,
                            )
                            nc.scalar.activation(
                                ost[:, eh * NB : (eh + 1) * NB],
                                ps[:],
                                mybir.ActivationFunctionType.Copy,
                                scale=OUT_SCALE,
                            )
                        nc.sync.dma_start(
                            out=out_part[rb * 128 : (rb + 1) * 128, :],
                            in_=ost[:],
                        )

    nc.finalize()
    return nc


def prep_in_maps(x, W1a, W1b, mat2a, mat2b, w_out):
    xT = np.ascontiguousarray(x.reshape(R, D).T).astype(np.float16)
    ident = np.eye(128, dtype=np.float16)
    # chunked-AG k order: (h_rel, rank, d) -> head h = 2*rank + h_rel
    k_perm = np.array(
        [2 * rank + h_rel for h_rel in range(2) for rank in range(NCORES)]
    )
    in_maps = []
    for c in range(NCORES):
        heads = [2 * c, 2 * c + 1]
        W1b_c = W1b[128 * c : 128 * (c + 1), :]  # [128 out-cols, D]
        W1b_c_perm = (
            W1b_c.reshape(128, H, HD)[:, k_perm, :].reshape(128, D)
        )
        in_maps.append(
            {
                "xT": xT,
                "w1aT": np.ascontiguousarray(
                    W1a[128 * c : 128 * (c + 1), :].T
                ).astype(np.float16),
                "w1bT": np.ascontiguousarray(W1b_c_perm.T).astype(np.float16),
                "trilAT": np.stack(
                    [np.tril(mat2a[h]).T.astype(np.float16) for h in heads]
                ),
                "trilBT": np.stack(
                    [np.tril(mat2b[h]).T.astype(np.float16) for h in heads]
                ),
                "wout": w_out[heads].reshape(128, D).astype(np.float16),
                "ident": ident,
            }
        )
    return in_maps


def kernel(x, W1a, W1b, mat2a, mat2b, w_out):
    x = np.asarray(x, dtype=np.float32)
    W1a = np.asarray(W1a, dtype=np.float32)
    W1b = np.asarray(W1b, dtype=np.float32)
    mat2a = np.asarray(mat2a, dtype=np.float32)
    mat2b = np.asarray(mat2b, dtype=np.float32)
    w_out = np.asarray(w_out, dtype=np.float32)

    if "nc" not in _NC_CACHE:
        _NC_CACHE["nc"] = build_nc()
    nc = _NC_CACHE["nc"]

    in_maps = prep_in_maps(x, W1a, W1b, mat2a, mat2b, w_out)
    res = run_bass_kernel_spmd(nc, in_maps, core_ids=list(range(NCORES)))
    out = np.zeros((R, D), np.float32)
    for c in range(NCORES):
        out += res.results[c]["out_part"].astype(np.float32)
    out *= 1.0 / OUT_SCALE
    return out.reshape(B, L, D)


if __name__ == "__main__":
    rng = np.random.default_rng(0)
    inputs = {
        "x": rng.standard_normal((B, L, D), dtype=np.float32),
        "W1a": rng.standard_normal((D, D), dtype=np.float32) / D,
        "W1b": rng.standard_normal((D, D), dtype=np.float32) / D,
        "mat2a": rng.standard_normal((H, L, L), dtype=np.float32) / 32,
        "mat2b": rng.standard_normal((H, L, L), dtype=np.float32) / 32,
        "w_out": rng.standard_normal((H, HD, D), dtype=np.float32) / D,
    }
    out = kernel(**inputs)
    print("kernel ran, out shape", out.shape)
